# revision 3
# baseline (speedup 1.0000x reference)
"""Causal self-attention on 8 TRN2 NeuronCores.

Problem: x[2,2048,1024], wq/wk/wv/wo[1024,1024] (nn.Linear convention,
out = y @ W.T), H=16 heads, D=64, causal softmax, f32.

Sharding: tensor-parallel over heads x data-parallel over batch.
Core i handles batch b=i//4 and head group g=i%4 (4 heads each).
wq/wk/wv are split row-wise (output-feature) per head group; wo is
split column-wise; each core returns a partial output projection
out_partial[b] and the host sums the 4 partials per batch.

On-device layout is fully "feature-major" (transposed): the host passes
xT=x[b].T etc so every matmul sees its contraction dim on SBUF
partitions and no on-device transposes are needed. Attention uses the
S^T formulation: scores^T[k,q] blocks, exp via ScalarE (fused *SCALE),
row sums via a ones-column folded into the PV matmul's stationary
operand, and the 1/sum normalization via a DRAM-roundtrip broadcast.
All matmuls run in float32r (TF32-like, 1 cycle/row at N>=256).
"""

import sys

for _p in ("/opt/trn_rl_repo", "/root/.axon_site"):
    if _p not in sys.path:
        sys.path.insert(0, _p)

import numpy as np

import concourse.bass as bass
import concourse.mybir as mybir
import concourse.tile as tile
from concourse import bacc
from concourse.bass_utils import run_bass_kernel_spmd

B, T, C, H = 2, 2048, 1024, 16
DH = C // H            # 64 head dim
HG = 4                 # heads per core
GW = HG * DH           # 256 features per head group
NB = T // 128          # 16 key chunks
NS = T // 512          # 4 query spans
KC = C // 128          # 8 contraction chunks over C
SCALE = 1.0 / float(np.sqrt(DH))
N_CORES = 8

F32 = mybir.dt.float32
F32R = mybir.dt.float32r


def build_nc():
    nc = bacc.Bacc("TRN2", target_bir_lowering=False, debug=False,
                   num_devices=N_CORES)
    xT = nc.declare_dram_parameter("xT", [C, T], F32R, isOutput=False)
    wqT = nc.declare_dram_parameter("wqT", [C, GW], F32R, isOutput=False)
    wkT = nc.declare_dram_parameter("wkT", [C, GW], F32R, isOutput=False)
    wvT = nc.declare_dram_parameter("wvT", [C, GW], F32R, isOutput=False)
    woT = nc.declare_dram_parameter("woT", [GW, C], F32R, isOutput=False)
    outT = nc.declare_dram_parameter("outT", [C, T], F32, isOutput=True)
    r_dram = nc.dram_tensor("r_scratch", [HG, NS, 512], F32)

    with tile.TileContext(nc) as tc:
        with tc.tile_pool(name="pers", bufs=1) as pers:
            # ---- persistent SBUF tensors ----
            xts = []
            for i in range(KC):
                t = pers.tile([128, T], F32R, tag=f"xT{i}", name=f"xT{i}")
                nc.sync.dma_start(out=t, in_=xT[i * 128:(i + 1) * 128, :])
                xts.append(t)
            wq_t, wk_t, wv_t = [], [], []
            for name, src, lst in (("wq", wqT, wq_t), ("wk", wkT, wk_t),
                                   ("wv", wvT, wv_t)):
                for i in range(KC):
                    t = pers.tile([128, GW], F32R, tag=f"{name}{i}", name=f"{name}{i}")
                    nc.sync.dma_start(out=t, in_=src[i * 128:(i + 1) * 128, :])
                    lst.append(t)
            wo_t = []
            for j in range(2):
                t = pers.tile([128, C], F32R, tag=f"wo{j}", name=f"wo{j}")
                nc.sync.dma_start(out=t, in_=woT[j * 128:(j + 1) * 128, :])
                wo_t.append(t)

            qts = [pers.tile([128, T], F32R, tag=f"qT{m}", name=f"qT{m}") for m in range(2)]
            kts = [pers.tile([128, T], F32R, tag=f"kT{m}", name=f"kT{m}") for m in range(2)]
            yts = [pers.tile([128, T], F32R, tag=f"yT{m}", name=f"yT{m}") for m in range(2)]

            # additive causal mask for diagonal 128x128 strips of S^T:
            # keep (0) where kl <= ql i.e. col >= row, else -1e9
            mask = pers.tile([128, 128], F32, tag="mask", name="mask")
            nc.gpsimd.memset(mask, 0.0)
            nc.gpsimd.affine_select(
                out=mask, in_=mask, compare_op=mybir.AluOpType.is_ge,
                fill=-1e9, base=0, pattern=[[1, 128]], channel_multiplier=-1,
            )
            # ones [128, 4] in f32r for V's ones-columns
            ones4 = pers.tile([128, 4], F32R, tag="ones4", name="ones4")
            for j in range(4):
                nc.scalar.activation(
                    out=ones4[:, j:j + 1], in_=nc.const_aps.tensor(1.0, [128, 1]),
                    func=mybir.ActivationFunctionType.Copy)

            # ---- phase 1: projections ----
            vts = [pers.tile([128, HG * 65], F32R, tag=f"V{tb}", name=f"V{tb}")
                   for tb in range(NB)]
            with tc.tile_pool(name="pp1", bufs=6, space="PSUM") as pp1, \
                 tc.tile_pool(name="pp2", bufs=2, space="PSUM") as pp2:
                # kT and qT first (attention consumes them earliest)
                for wt, dest in ((wk_t, kts), (wq_t, qts)):
                    for m in range(2):
                        pss = [pp1.tile([128, 512], F32, tag="projps", name="projps")
                               for _ in range(NS)]
                        for k in range(KC):
                            for s in range(NS):
                                nc.tensor.matmul(
                                    pss[s],
                                    wt[k][:, m * 128:(m + 1) * 128],
                                    xts[k][:, s * 512:(s + 1) * 512],
                                    start=(k == 0), stop=(k == KC - 1))
                        for s in range(NS):
                            nc.vector.tensor_copy(
                                out=dest[m][:, s * 512:(s + 1) * 512],
                                in_=pss[s])
                # V in natural [t, d] layout: stationary = xT chunk
                for tb in range(NB):
                    vps = pp2.tile([128, GW], F32, tag="vps", name="vps")
                    for k in range(KC):
                        nc.tensor.matmul(
                            vps, xts[k][:, tb * 128:(tb + 1) * 128], wv_t[k],
                            start=(k == 0), stop=(k == KC - 1))
                    vt = vts[tb]
                    for h in range(HG):
                        nc.vector.tensor_copy(
                            out=vt[:, h * 65:h * 65 + 64],
                            in_=vps[:, h * 64:(h + 1) * 64])
                    nc.vector.tensor_copy(
                        out=vt.rearrange("p (h c) -> p h c", c=65)[:, :, 64],
                        in_=ones4)

            # ---- phase 2: attention, head by head ----
            with tc.tile_pool(name="sps", bufs=3, space="PSUM") as sps, \
                 tc.tile_pool(name="pvs", bufs=4, space="PSUM") as pvs, \
                 tc.tile_pool(name="ptp", bufs=4) as ptp, \
                 tc.tile_pool(name="rp", bufs=3) as rp:
                for h in range(HG):
                    qt, kt, yt = qts[h // 2], kts[h // 2], yts[h // 2]
                    po = (h % 2) * 64
                    pv = [pvs.tile([65, 512], F32, tag="pv", name="pv") for _ in range(NS)]
                    for ki in range(NB):
                        for s in range(ki // 4, NS):
                            c0 = 128 * (ki - 4 * s) if ki >= 4 * s else 0
                            sp = sps.tile([128, 512], F32, tag="sp", name="sp")
                            nc.tensor.matmul(
                                sp[:, c0:],
                                kt[po:po + 64, ki * 128:(ki + 1) * 128],
                                qt[po:po + 64, s * 512 + c0:(s + 1) * 512],
                                start=True, stop=True)
                            if ki >= 4 * s:  # diagonal chunk: mask strip
                                nc.vector.tensor_add(
                                    out=sp[:, c0:c0 + 128],
                                    in0=sp[:, c0:c0 + 128], in1=mask)
                            pt = ptp.tile([128, 512], F32R, tag="pt", name="pt")
                            nc.scalar.activation(
                                out=pt[:, c0:], in_=sp[:, c0:],
                                func=mybir.ActivationFunctionType.Exp,
                                scale=SCALE)
                            nc.tensor.matmul(
                                pv[s][:, c0:],
                                vts[ki][:, h * 65:(h + 1) * 65],
                                pt[:, c0:],
                                start=(ki == 0), stop=(ki == 4 * s + 3))
                    for s in range(NS):
                        r1 = rp.tile([1, 512], F32, tag="r1", name="r1")
                        nc.vector.reciprocal(out=r1, in_=pv[s][64:65, :])
                        nc.sync.dma_start(out=r_dram[h, s, :], in_=r1)
                        rb = rp.tile([64, 512], F32, tag="rb", name="rb")
                        rsl = r_dram[h, s, :]
                        nc.sync.dma_start(
                            out=rb,
                            in_=bass.AP(tensor=rsl.tensor, offset=rsl.offset,
                                        ap=[[0, 64]] + list(rsl.ap)))
                        nc.vector.tensor_mul(
                            out=yt[po:po + 64, s * 512:(s + 1) * 512],
                            in0=pv[s][0:64, :], in1=rb)

            # ---- phase 3: output projection (partial sums) ----
            with tc.tile_pool(name="ops", bufs=4, space="PSUM") as ops, \
                 tc.tile_pool(name="ost", bufs=4) as ost:
                for m in range(8):
                    for s in range(NS):
                        op = ops.tile([128, 512], F32, tag="op", name="op")
                        for j in range(2):
                            nc.tensor.matmul(
                                op,
                                wo_t[j][:, m * 128:(m + 1) * 128],
                                yts[j][:, s * 512:(s + 1) * 512],
                                start=(j == 0), stop=(j == 1))
                        ot = ost.tile([128, 512], F32, tag="ot", name="ot")
                        nc.vector.tensor_copy(out=ot, in_=op)
                        nc.sync.dma_start(
                            out=outT[m * 128:(m + 1) * 128,
                                     s * 512:(s + 1) * 512],
                            in_=ot)
    nc.compile()
    return nc


_NC_CACHE = None


def _get_nc():
    global _NC_CACHE
    if _NC_CACHE is None:
        _NC_CACHE = build_nc()
    return _NC_CACHE


def make_in_maps(x, wq, wk, wv, wo):
    x = np.asarray(x, dtype=np.float32)
    wq = np.asarray(wq, dtype=np.float32)
    wk = np.asarray(wk, dtype=np.float32)
    wv = np.asarray(wv, dtype=np.float32)
    wo = np.asarray(wo, dtype=np.float32)
    in_maps = []
    for core in range(N_CORES):
        b, g = core // HG, core % HG
        rows = slice(g * GW, (g + 1) * GW)
        in_maps.append({
            "xT": np.ascontiguousarray(x[b].T),
            "wqT": np.ascontiguousarray(wq[rows, :].T),
            "wkT": np.ascontiguousarray(wk[rows, :].T),
            "wvT": np.ascontiguousarray(wv[rows, :].T),
            "woT": np.ascontiguousarray(wo[:, rows].T),
        })
    return in_maps


def run(x, wq, wk, wv, wo, trace=False, tmpdir=None):
    nc = _get_nc()
    in_maps = make_in_maps(x, wq, wk, wv, wo)
    res = run_bass_kernel_spmd(nc, in_maps, core_ids=list(range(N_CORES)),
                               trace=trace, tmpdir=tmpdir)
    out = np.zeros((B, T, C), dtype=np.float32)
    for core in range(N_CORES):
        out[core // HG] += res.results[core]["outT"].T
    return out, res


def kernel(x, wq, wk, wv, wo):
    out, _ = run(x, wq, wk, wv, wo)
    return out


# revision 6
# speedup vs baseline: 1.1210x; 1.1210x over previous
"""Causal self-attention on 8 TRN2 NeuronCores.

Problem: x[2,2048,1024], wq/wk/wv/wo[1024,1024] (nn.Linear convention,
out = y @ W.T), H=16 heads, D=64, causal softmax, f32.

Sharding: tensor-parallel over heads x data-parallel over batch.
Core i handles batch b=i//4 and head group g=i%4 (4 heads each).
wq/wk/wv are split row-wise (output-feature) per head group; wo is
split column-wise; each core returns a partial output projection
out_partial[b] and the host sums the 4 partials per batch.

On-device layout is fully "feature-major" (transposed): the host passes
xT=x[b].T etc so every matmul sees its contraction dim on SBUF
partitions and no on-device transposes are needed. Attention uses the
S^T formulation: scores^T[k,q] blocks, exp via ScalarE (fused *SCALE),
row sums via a ones-column folded into the PV matmul's stationary
operand, and the 1/sum normalization via a DRAM-roundtrip broadcast.
All matmuls run in float32r (TF32-like, 1 cycle/row at N>=256).
"""

import sys

for _p in ("/opt/trn_rl_repo", "/root/.axon_site"):
    if _p not in sys.path:
        sys.path.insert(0, _p)

import numpy as np

import concourse.bass as bass
import concourse.mybir as mybir
import concourse.tile as tile
from concourse import bacc
from concourse.bass_utils import run_bass_kernel_spmd

B, T, C, H = 2, 2048, 1024, 16
DH = C // H            # 64 head dim
HG = 4                 # heads per core
GW = HG * DH           # 256 features per head group
NB = T // 128          # 16 key chunks
NS = T // 512          # 4 query spans
KC = C // 128          # 8 contraction chunks over C
SCALE = 1.0 / float(np.sqrt(DH))
N_CORES = 8

F32 = mybir.dt.float32
F32R = mybir.dt.float32r
BF16 = mybir.dt.bfloat16


def build_nc():
    nc = bacc.Bacc("TRN2", target_bir_lowering=False, debug=False,
                   num_devices=N_CORES)
    xT = nc.declare_dram_parameter("xT", [C, T], F32R, isOutput=False)
    wqT = nc.declare_dram_parameter("wqT", [C, GW], F32R, isOutput=False)
    wkT = nc.declare_dram_parameter("wkT", [C, GW], F32R, isOutput=False)
    wvT = nc.declare_dram_parameter("wvT", [C, GW], F32R, isOutput=False)
    woT = nc.declare_dram_parameter("woT", [GW, C], F32R, isOutput=False)
    outT = nc.declare_dram_parameter("outT", [C, T], F32, isOutput=True)
    r_dram = nc.dram_tensor("r_scratch", [HG, NS, 512], F32)

    with tile.TileContext(nc) as tc:
        with tc.tile_pool(name="pers", bufs=1) as pers:
            # ---- persistent SBUF tensors ----
            xts = []
            for i in range(KC):
                t = pers.tile([128, T], F32R, tag=f"xT{i}", name=f"xT{i}")
                nc.sync.dma_start(out=t, in_=xT[i * 128:(i + 1) * 128, :])
                xts.append(t)
            wq_t, wk_t, wv_t = [], [], []
            for name, src, lst in (("wq", wqT, wq_t), ("wk", wkT, wk_t),
                                   ("wv", wvT, wv_t)):
                for i in range(KC):
                    t = pers.tile([128, GW], F32R, tag=f"{name}{i}", name=f"{name}{i}")
                    nc.sync.dma_start(out=t, in_=src[i * 128:(i + 1) * 128, :])
                    lst.append(t)
            wo_t = []
            for j in range(2):
                t = pers.tile([128, C], F32R, tag=f"wo{j}", name=f"wo{j}")
                nc.sync.dma_start(out=t, in_=woT[j * 128:(j + 1) * 128, :])
                wo_t.append(t)

            qts = [pers.tile([128, T], F32R, tag=f"qT{m}", name=f"qT{m}") for m in range(2)]
            kts = [pers.tile([128, T], F32R, tag=f"kT{m}", name=f"kT{m}") for m in range(2)]
            yts = [pers.tile([128, T], F32R, tag=f"yT{m}", name=f"yT{m}") for m in range(2)]

            # additive causal mask for diagonal 128x128 strips of S^T:
            # keep (0) where kl <= ql i.e. col >= row, else -1e9
            mask = pers.tile([128, 128], F32, tag="mask", name="mask")
            nc.gpsimd.memset(mask, 0.0)
            nc.gpsimd.affine_select(
                out=mask, in_=mask, compare_op=mybir.AluOpType.is_ge,
                fill=-1e9, base=0, pattern=[[1, 128]], channel_multiplier=-1,
            )
            # all-masked strip (every k > q) for over-computed columns
            maskf = pers.tile([128, 128], F32, tag="maskf", name="maskf")
            nc.gpsimd.memset(maskf, -1e9)
            # ones [128, 4] in bf16 for V's ones-columns
            ones4 = pers.tile([128, 4], BF16, tag="ones4", name="ones4")
            for j in range(4):
                nc.scalar.activation(
                    out=ones4[:, j:j + 1], in_=nc.const_aps.tensor(1.0, [128, 1]),
                    func=mybir.ActivationFunctionType.Copy)

            # ---- phase 1: projections ----
            vts = [pers.tile([128, HG * 65], BF16, tag=f"V{tb}", name=f"V{tb}")
                   for tb in range(NB)]
            with tc.tile_pool(name="pp1", bufs=6, space="PSUM") as pp1, \
                 tc.tile_pool(name="pp2", bufs=2, space="PSUM") as pp2:
                # kT and qT first (attention consumes them earliest)
                for wt, dest in ((wk_t, kts), (wq_t, qts)):
                    for m in range(2):
                        pss = [pp1.tile([128, 512], F32, tag="projps", name="projps")
                               for _ in range(NS)]
                        for k in range(KC):
                            for s in range(NS):
                                nc.tensor.matmul(
                                    pss[s],
                                    wt[k][:, m * 128:(m + 1) * 128],
                                    xts[k][:, s * 512:(s + 1) * 512],
                                    start=(k == 0), stop=(k == KC - 1))
                        for s in range(NS):
                            nc.vector.tensor_copy(
                                out=dest[m][:, s * 512:(s + 1) * 512],
                                in_=pss[s])
                # V in natural [t, d] layout: stationary = xT chunk
                for tb in range(NB):
                    vps = pp2.tile([128, GW], F32, tag="vps", name="vps")
                    for k in range(KC):
                        nc.tensor.matmul(
                            vps, xts[k][:, tb * 128:(tb + 1) * 128], wv_t[k],
                            start=(k == 0), stop=(k == KC - 1))
                    vt = vts[tb]
                    for h in range(HG):
                        nc.vector.tensor_copy(
                            out=vt[:, h * 65:h * 65 + 64],
                            in_=vps[:, h * 64:(h + 1) * 64])
                    nc.vector.tensor_copy(
                        out=vt.rearrange("p (h c) -> p h c", c=65)[:, :, 64],
                        in_=ones4)

            # ---- phase 2: attention, head by head, span by span ----
            # Two-pass per span keeps the PE instruction stream dense:
            # all scores for the span first (exp trails on ScalarE), then
            # the PV accumulation reading the bf16 P^T tiles.
            with tc.tile_pool(name="sps", bufs=4, space="PSUM") as sps, \
                 tc.tile_pool(name="pvs", bufs=2, space="PSUM") as pvs, \
                 tc.tile_pool(name="ptp", bufs=20) as ptp, \
                 tc.tile_pool(name="rp", bufs=3) as rp:
                for h in range(HG):
                    qt, kt, yt = qts[h // 2], kts[h // 2], yts[h // 2]
                    po = (h % 2) * 64
                    for s in range(NS):
                        nki = 4 * s + 4
                        pts = []
                        for ki in range(nki):
                            c0 = 128 * (ki - 4 * s) if ki >= 4 * s else 0
                            c0 = min(c0, 256)   # keep f32r moving dim >= 256
                            sp = sps.tile([128, 512], F32, tag="sp", name="sp")
                            nc.tensor.matmul(
                                sp[:, c0:],
                                kt[po:po + 64, ki * 128:(ki + 1) * 128],
                                qt[po:po + 64, s * 512 + c0:(s + 1) * 512],
                                start=True, stop=True)
                            d0 = 128 * (ki - 4 * s) if ki >= 4 * s else -1
                            if d0 == 384:  # over-computed fully-invalid strip
                                nc.vector.tensor_add(
                                    out=sp[:, 256:384],
                                    in0=sp[:, 256:384], in1=maskf)
                            if d0 >= 0:   # diagonal strip
                                nc.vector.tensor_add(
                                    out=sp[:, d0:d0 + 128],
                                    in0=sp[:, d0:d0 + 128], in1=mask)
                            pt = ptp.tile([128, 512], BF16, tag="pt", name="pt")
                            nc.scalar.activation(
                                out=pt[:, c0:], in_=sp[:, c0:],
                                func=mybir.ActivationFunctionType.Exp,
                                scale=SCALE)
                            pts.append((pt, c0))
                        pv = pvs.tile([65, 512], F32, tag="pv", name="pv")
                        for ki in range(nki):
                            pt, c0 = pts[ki]
                            nc.tensor.matmul(
                                pv[:, c0:],
                                vts[ki][:, h * 65:(h + 1) * 65],
                                pt[:, c0:],
                                start=(ki == 0), stop=(ki == nki - 1))
                        r1 = rp.tile([1, 512], F32, tag="r1", name="r1")
                        nc.vector.reciprocal(out=r1, in_=pv[64:65, :])
                        nc.sync.dma_start(out=r_dram[h, s, :], in_=r1)
                        rb = rp.tile([64, 512], F32, tag="rb", name="rb")
                        rsl = r_dram[h, s, :]
                        nc.sync.dma_start(
                            out=rb,
                            in_=bass.AP(tensor=rsl.tensor, offset=rsl.offset,
                                        ap=[[0, 64]] + list(rsl.ap)))
                        nc.vector.tensor_mul(
                            out=yt[po:po + 64, s * 512:(s + 1) * 512],
                            in0=pv[0:64, :], in1=rb)

            # ---- phase 3: output projection (partial sums) ----
            with tc.tile_pool(name="ops", bufs=4, space="PSUM") as ops, \
                 tc.tile_pool(name="ost", bufs=4) as ost:
                for m in range(8):
                    for s in range(NS):
                        op = ops.tile([128, 512], F32, tag="op", name="op")
                        for j in range(2):
                            nc.tensor.matmul(
                                op,
                                wo_t[j][:, m * 128:(m + 1) * 128],
                                yts[j][:, s * 512:(s + 1) * 512],
                                start=(j == 0), stop=(j == 1))
                        ot = ost.tile([128, 512], F32, tag="ot", name="ot")
                        nc.vector.tensor_copy(out=ot, in_=op)
                        nc.sync.dma_start(
                            out=outT[m * 128:(m + 1) * 128,
                                     s * 512:(s + 1) * 512],
                            in_=ot)
    nc.compile()
    return nc


_NC_CACHE = None


def _get_nc():
    global _NC_CACHE
    if _NC_CACHE is None:
        _NC_CACHE = build_nc()
    return _NC_CACHE


def make_in_maps(x, wq, wk, wv, wo):
    x = np.asarray(x, dtype=np.float32)
    wq = np.asarray(wq, dtype=np.float32)
    wk = np.asarray(wk, dtype=np.float32)
    wv = np.asarray(wv, dtype=np.float32)
    wo = np.asarray(wo, dtype=np.float32)
    in_maps = []
    for core in range(N_CORES):
        b, g = core // HG, core % HG
        rows = slice(g * GW, (g + 1) * GW)
        in_maps.append({
            "xT": np.ascontiguousarray(x[b].T),
            "wqT": np.ascontiguousarray(wq[rows, :].T),
            "wkT": np.ascontiguousarray(wk[rows, :].T),
            "wvT": np.ascontiguousarray(wv[rows, :].T),
            "woT": np.ascontiguousarray(wo[:, rows].T),
        })
    return in_maps


def run(x, wq, wk, wv, wo, trace=False, tmpdir=None):
    nc = _get_nc()
    in_maps = make_in_maps(x, wq, wk, wv, wo)
    res = run_bass_kernel_spmd(nc, in_maps, core_ids=list(range(N_CORES)),
                               trace=trace, tmpdir=tmpdir)
    out = np.zeros((B, T, C), dtype=np.float32)
    for core in range(N_CORES):
        out[core // HG] += res.results[core]["outT"].T
    return out, res


def kernel(x, wq, wk, wv, wo):
    out, _ = run(x, wq, wk, wv, wo)
    return out
